# revision 29
# baseline (speedup 1.0000x reference)
"""Causal self-attention (per-head full-D k/q, DH-wide v) on 8 trn2 cores.

Sharding: tensor-parallel over heads. Core c owns heads (2c, 2c+1).

Key algebraic restructure vs the straightforward kernel: k and q are only
ever used through q @ k^T, so per head the two projections fold into one
matrix  M = Wq @ Wk^T  (computed once on device) and

  scores^T = x @ (x @ M)^T / sqrt(D)

replaces the 2*D-wide k/q projection (per batch) with a D-wide t = x@M
projection plus a once-per-head M precompute. All three matmuls in that
chain (M precompute, t = x@M, scores = x@t^T) run in fp8e4 with DoubleRow
perf mode (2 contraction tiles per instruction, ~2x PE rate): weights and
M are scaled by 64 so their values sit in e4m3's normal range; the exp()
activation scale folds the 1/64 back out along with the 1/sqrt(D).
Measured scale-relative absmax error 1.67e-2 (gate 2e-2); the all-bf16
fallback (MODE=0) measures 4.1e-3.

Per core, for all 4 batches:
  M[h]    = Wq8[h] @ Wk8[h]^T          (fp8 DR, f32 psum -> fp8*64)
  t^T     = M-contraction with x8^T    (fp8 DR, f32 psum -> fp8)
  S^T     = x8 @ t8^T                  (fp8 DR, 512-wide n blocks; causal:
                                        12 of 16 tiles, diagonal tiles
                                        column-trimmed to the valid range)
  P^T     = exp(S^T / 2048) * mask     (ACT exp -> bf16; 8 masked tiles)
  O^T_aug = [v | 1]^T @ P^T            (bf16; row 64 = softmax denominator)
  ostack  = O^T / den                  (DVE fast-reciprocal on the 1-row
                                        den, gpsimd partition_broadcast,
                                        DVE mul; ACT evacuates O)
  partial = ostack.T @ Wp[128c:+128]   (bf16; partials DMA'd out in bf16)
Host sums the 8 partials and adds bp.

v is computed directly in [tokens, v-col] orientation (lhsT = x tiles) so
no transpose is needed; both heads' 64 v columns per 128-wide matmul.

Scheduling: 16 warmup matmuls ramp the PE clock during the DMA lead-in;
M[h0], b0's v-projection, and M[h1] interleave with the staging DMAs; the
next batch's x is prefetched a full batch ahead; deferred A@v tails and
the previous batch's output projection drain one-per-tile inside the next
t-projection phase so the PE never waits on the exp (ACT) or normalize
(DVE/Pool) chains. PSUM evacuations alternate DVE/ACT (gpsimd has no
PSUM port and keeps a single ucode library loaded: partition_broadcast).

NOTE: bkqv is all-zeros per the problem spec and is not applied (the k/q
part would need rank-1 score corrections, the v part a per-column add).
"""

import sys
import types

import numpy as np
import ml_dtypes

import concourse.bass as bass
import concourse.bacc as bacc
import concourse.tile as tile
from concourse import mybir
from concourse.bass_utils import run_bass_kernel_spmd

# If BASS_TRACE is set in the environment, run_bass_kernel_spmd imports
# antenv.axon_hooks, which this image may not ship. Register a stub that
# reports "no hook" so tracing degrades gracefully instead of crashing.
try:
    from antenv.axon_hooks import get_axon_ntff_profile_hook  # noqa: F401
except ImportError:
    import antenv

    _mod = types.ModuleType("antenv.axon_hooks")
    _mod.get_axon_ntff_profile_hook = lambda: None
    _mod.set_axon_ntff_profile_hook = lambda h: setattr(
        _mod, "get_axon_ntff_profile_hook", lambda: h
    )
    antenv.axon_hooks = _mod
    sys.modules["antenv.axon_hooks"] = _mod

BF16 = mybir.dt.bfloat16
F32 = mybir.dt.float32
F8 = mybir.dt.float8e4
AF = mybir.ActivationFunctionType
DR = mybir.MatmulPerfMode.DoubleRow

B, N, D, H, DH = 4, 1024, 1024, 16, 64
P = 128
NCORES = 8
HL = H // NCORES        # 2 local heads per core
DT = D // P             # 8 contraction tiles
DP = DT // 2            # 4 DoubleRow contraction pairs
NJ = N // 512           # 2 512-wide n blocks
NT = N // P             # 8 token tiles
VG = 96                 # v group stride ([v(64) | ones(1)] padded)
MSC = 64.0              # fp8 scale baked into M (and t, scores)
EXPSCALE = 1.0 / (32.0 * MSC)

# numeric mode: 3 = also fp8 M-precompute, 2 = fp8 t-proj + fp8 scores
# (DoubleRow), 1 = fp8 t-proj only, 0 = all-bf16 (structure validation)
MODE = 3

_CACHE = {}


def _build_nc():
    nc = bacc.Bacc(
        "TRN2",
        target_bir_lowering=False,
        debug=False,
        enable_asserts=True,
        num_devices=NCORES,
    )
    xt_d = nc.declare_dram_parameter("xt", [B, D, N], BF16, isOutput=False)
    x8_d = nc.declare_dram_parameter("xt8", [B, D, N], F8, isOutput=False)
    W_DT = F8 if MODE >= 3 else BF16
    wq_d = nc.declare_dram_parameter("wqT", [HL, D, D], W_DT, isOutput=False)
    wk_d = nc.declare_dram_parameter("wkT", [HL, D, D], W_DT, isOutput=False)
    wv_d = nc.declare_dram_parameter("wv", [D, HL * DH], BF16, isOutput=False)
    wp_d = nc.declare_dram_parameter("wp", [P, D], BF16, isOutput=False)
    mask_d = nc.declare_dram_parameter("masks", [4, P, 512], BF16, isOutput=False)
    out_d = nc.declare_dram_parameter("out", [B, N, D], BF16, isOutput=True)

    with tile.TileContext(nc) as tc:
        with (
            tc.tile_pool(name="const", bufs=1) as constp,
            tc.tile_pool(name="mpool", bufs=1) as mpool,
            tc.tile_pool(name="stagep", bufs=(2 if MODE >= 1 else 1)) as stagep,
            tc.tile_pool(name="xpool", bufs=2) as xpool,
            tc.tile_pool(name="x8pool", bufs=2) as x8pool,
            tc.tile_pool(name="ttpool", bufs=2) as ttpool,
            tc.tile_pool(name="vpool", bufs=2) as vpool,
            tc.tile_pool(name="ptpool", bufs=10) as ptpool,
            tc.tile_pool(name="otpool", bufs=2) as otpool,
            tc.tile_pool(name="stpool", bufs=3) as stpool,
            tc.tile_pool(name="pspool", bufs=1, space="PSUM") as pspool,
        ):
            # ---- resident constants ----
            wv_sb = constp.tile([P, DT * HL * DH], BF16, name="wv_sb")
            wp_sb = constp.tile([P, D], BF16, name="wp_sb")
            mask_sb = constp.tile([P, 4 * 512], BF16, name="mask_sb")
            # per-head M scaled by MSC: [d-in-tile, d-tile, d-out]
            M_DT = F8 if MODE >= 1 else BF16
            TT_DT = F8 if MODE >= 2 else BF16
            m8 = [
                mpool.tile([P, DT, D], M_DT, name=f"m8_{h}") for h in range(HL)
            ]

            def emit_vproj(xt_sb):
                # packed v projection, computed directly in [tokens, v-cols]
                # orientation (lhsT = x tiles), so no transpose is needed
                v_sb = vpool.tile([P, HL * NT * VG], BF16, tag="vaug", name="v_sb")
                nc.vector.memset(v_sb[:, :], 1.0)
                for i in range(NT):
                    ps_v = pspool.tile([P, P], F32, tag="ps", bufs=4, name="ps_v")
                    for d in range(DT):
                        nc.tensor.matmul(
                            ps_v[:],
                            lhsT=xt_sb[:, d, i * P:(i + 1) * P],
                            rhs=wv_sb[:, d * P:(d + 1) * P],
                            start=(d == 0),
                            stop=(d == DT - 1),
                        )
                    for hh in range(HL):
                        dst = v_sb[:, (hh * NT + i) * VG:(hh * NT + i) * VG + DH]
                        if (i + hh) % 2 == 0:
                            nc.vector.tensor_copy(dst, ps_v[:, hh * DH:(hh + 1) * DH])
                        else:
                            nc.scalar.activation(dst, ps_v[:, hh * DH:(hh + 1) * DH], AF.Copy)
                return v_sb

            def fetch_x(b, xt_sb=None, x8_sb=None):
                if xt_sb is None:
                    xt_sb = xpool.tile([P, DT, N], BF16, tag="xt", name="xt_sb")
                    for d in range(DT):
                        nc.sync.dma_start(
                            out=xt_sb[:, d, :], in_=xt_d[b, d * P:(d + 1) * P, :]
                        )
                if x8_sb is None:
                    x8_sb = x8pool.tile([P, DT, N], F8, tag="x8", name="x8_sb")
                    for d in range(DT):
                        nc.sync.dma_start(
                            out=x8_sb[:, d, :], in_=x8_d[b, d * P:(d + 1) * P, :]
                        )
                return xt_sb, x8_sb

            # ---- PE warmup: ramp the clock while the first DMAs land ----
            warm = constp.tile([P, 512], BF16, name="warm")
            nc.vector.memset(warm[:, :], 0.0)
            for k in range(16):
                ps_w = pspool.tile([P, 512], F32, tag="ps", bufs=4, name="ps_w")
                nc.tensor.matmul(
                    ps_w[:], lhsT=warm[:, 0:P], rhs=warm[:], start=True, stop=True
                )

            # ---- M precompute: per head, M = Wq @ Wk^T via transposed
            # weight tiles (contraction over e on partitions). b0's v
            # projection runs between the two heads, covering head 1's
            # staging DMA. ----
            for h in range(HL):
                wqs = stagep.tile([P, DT, D], W_DT, tag="wqs", name="wqs")
                wks = stagep.tile([P, DT, D], W_DT, tag="wks", name="wks")
                for e in range(DT):
                    nc.sync.dma_start(
                        out=wqs[:, e, :], in_=wq_d[h, e * P:(e + 1) * P, :]
                    )
                    nc.sync.dma_start(
                        out=wks[:, e, :], in_=wk_d[h, e * P:(e + 1) * P, :]
                    )
                if h == 0:
                    # b0's v-projection inputs, right behind h0's staging
                    for dd in range(DT):
                        nc.sync.dma_start(
                            out=wv_sb[:, dd * P:(dd + 1) * P],
                            in_=wv_d[dd * P:(dd + 1) * P, :],
                        )
                    xt_sb0, _ = fetch_x(0, x8_sb="later")
                else:
                    _, x8_sb0 = fetch_x(0, xt_sb="later")
                    for m in range(4):
                        nc.sync.dma_start(
                            out=mask_sb[:, m * 512:(m + 1) * 512], in_=mask_d[m]
                        )
                    nc.sync.dma_start(out=wp_sb[:], in_=wp_d[:])
                for dd in range(DT):
                    for jm in range(NJ):
                        ps_m = pspool.tile([P, 512], F32, tag="ps", bufs=4, name="ps_m")
                        if MODE >= 3:
                            # host pre-scales Wq/Wk by 64 -> psum holds
                            # 4096*M; rescale to MSC*M on evacuation
                            for ep in range(DP):
                                nc.tensor.matmul(
                                    ps_m[:],
                                    lhsT=wqs[:, 2 * ep:2 * ep + 2, dd * P:(dd + 1) * P],
                                    rhs=wks[:, 2 * ep:2 * ep + 2, jm * 512:(jm + 1) * 512],
                                    start=(ep == 0),
                                    stop=(ep == DP - 1),
                                    perf_mode=DR,
                                )
                        else:
                            for e in range(DT):
                                nc.tensor.matmul(
                                    ps_m[:],
                                    lhsT=wqs[:, e, dd * P:(dd + 1) * P],
                                    rhs=wks[:, e, jm * 512:(jm + 1) * 512],
                                    start=(e == 0),
                                    stop=(e == DT - 1),
                                )
                        nc.scalar.activation(
                            m8[h][:, dd, jm * 512:(jm + 1) * 512],
                            ps_m[:],
                            AF.Copy,
                            scale=(MSC / (MSC * MSC) if MODE >= 3 else MSC),
                        )
                if h == 0:
                    # b0's v projection covers head 1's staging DMA wait
                    v_sb0 = emit_vproj(xt_sb0)

            # ---- deferred emissions keep the PE fed ----
            pending = []  # list of closures, interleaved into t-proj phases

            def emit_outproj(bb, ost):
                insts = []
                for t in range(NT):
                    for j2 in range(NJ):
                        def mk(t=t, j2=j2, bb=bb, ost=ost):
                            ps_f = pspool.tile(
                                [P, 512], F32, tag="psf", bufs=2, name="ps_f"
                            )
                            nc.tensor.matmul(
                                ps_f[:],
                                lhsT=ost[:, t * P:(t + 1) * P],
                                rhs=wp_sb[:, j2 * 512:(j2 + 1) * 512],
                                start=True, stop=True,
                            )
                            stage = stpool.tile(
                                [P, 512], BF16, tag="stage", name="stage"
                            )
                            if (t + j2) % 2 == 0:
                                nc.scalar.activation(stage[:], ps_f[:], AF.Copy)
                            else:
                                nc.vector.tensor_copy(stage[:], ps_f[:])
                            nc.sync.dma_start(
                                out=out_d[
                                    bb, t * P:(t + 1) * P,
                                    j2 * 512:(j2 + 1) * 512,
                                ],
                                in_=stage[:],
                            )
                        insts.append(mk)
                return insts

            def drain_pending(k):
                for _ in range(k):
                    if pending:
                        pending.pop(0)()

            next_x = (xt_sb0, x8_sb0)
            for b in range(B):
                xt_sb, x8_sb = next_x
                if b + 1 < B:
                    next_x = fetch_x(b + 1)

                if b == 0:
                    v_sb = v_sb0  # emitted during the M-precompute phase
                else:
                    v_sb = emit_vproj(xt_sb)

                ostack = otpool.tile([P, N], BF16, tag="ostack", name="ostack")

                for h in range(HL):
                    # ---- t projection: t^T[d'-tile, n] ----
                    tt8 = ttpool.tile([P, DT, N], TT_DT, tag="tt", name="tt8")
                    nmm = 0
                    for jh in range(NJ):
                        for dd in range(DT):
                            ps_y = pspool.tile(
                                [P, 512], F32, tag="ps", bufs=4, name="ps_y"
                            )
                            if MODE >= 1:
                                for dp in range(DP):
                                    nc.tensor.matmul(
                                        ps_y[:],
                                        lhsT=m8[h][:, 2 * dp:2 * dp + 2, dd * P:(dd + 1) * P],
                                        rhs=x8_sb[:, 2 * dp:2 * dp + 2, jh * 512:(jh + 1) * 512],
                                        start=(dp == 0),
                                        stop=(dp == DP - 1),
                                        perf_mode=DR,
                                    )
                            else:
                                for d in range(DT):
                                    nc.tensor.matmul(
                                        ps_y[:],
                                        lhsT=m8[h][:, d, dd * P:(dd + 1) * P],
                                        rhs=xt_sb[:, d, jh * 512:(jh + 1) * 512],
                                        start=(d == 0),
                                        stop=(d == DT - 1),
                                    )
                            # evacuate psum alternating DVE/ACT (gpsimd has
                            # no PSUM port)
                            if dd % 2 == 0:
                                nc.vector.tensor_copy(
                                    tt8[:, dd, jh * 512:(jh + 1) * 512], ps_y[:]
                                )
                            else:
                                nc.scalar.activation(
                                    tt8[:, dd, jh * 512:(jh + 1) * 512],
                                    ps_y[:], AF.Copy,
                                )
                            nmm += 1
                            if nmm >= 5:
                                drain_pending(1)

                    # ---- scores + A@v per 512-wide n block ----
                    ps_os = {}

                    def mk_score(i, j, h=h, tt8=tt8, x8_sb=x8_sb, xt_sb=xt_sb):
                        def go():
                            # diagonal tiles: the first mi*128 columns are
                            # fully masked -- skip computing them entirely
                            # (A@v consumes the matching sub-range only)
                            mi = i - 4 * j
                            c0 = mi * P if mi > 0 else 0
                            ps_s = pspool.tile(
                                [P, 512], F32, tag="ps", bufs=4, name="ps_s"
                            )
                            if MODE >= 2:
                                for dp in range(DP):
                                    nc.tensor.matmul(
                                        ps_s[:, c0:],
                                        lhsT=x8_sb[:, 2 * dp:2 * dp + 2, i * P:(i + 1) * P],
                                        rhs=tt8[:, 2 * dp:2 * dp + 2, j * 512 + c0:(j + 1) * 512],
                                        start=(dp == 0),
                                        stop=(dp == DP - 1),
                                        perf_mode=DR,
                                    )
                            else:
                                for d in range(DT):
                                    nc.tensor.matmul(
                                        ps_s[:, c0:],
                                        lhsT=xt_sb[:, d, i * P:(i + 1) * P],
                                        rhs=tt8[:, d, j * 512 + c0:(j + 1) * 512],
                                        start=(d == 0),
                                        stop=(d == DT - 1),
                                    )
                            pt = ptpool.tile([P, 512], BF16, tag="pt", name="pt")
                            nc.scalar.activation(
                                pt[:, c0:], ps_s[:, c0:], AF.Exp, scale=EXPSCALE
                            )
                            if mi >= 0:  # diagonal-crossing tile
                                pt2 = ptpool.tile(
                                    [P, 512], BF16, tag="pt2", bufs=6, name="pt2"
                                )
                                nc.vector.tensor_mul(
                                    pt2[:, c0:], pt[:, c0:],
                                    mask_sb[:, mi * 512 + c0:(mi + 1) * 512],
                                )
                                return pt2, c0
                            return pt, c0
                        return go

                    def mk_av(i, j, nm, pt_get, h=h, b=b, ostack=ostack, v_sb=v_sb):
                        def go():
                            pt, c0 = pt_get()
                            if j not in ps_os:
                                ps_os[j] = pspool.tile(
                                    [DH + 1, 512], F32, tag="po", bufs=2, name="ps_o"
                                )
                            nc.tensor.matmul(
                                ps_os[j][:, c0:],
                                lhsT=v_sb[:, (h * NT + i) * VG:(h * NT + i) * VG + DH + 1],
                                rhs=pt[:, c0:],
                                start=(i == 0),
                                stop=(i == nm - 1),
                            )
                            if i == nm - 1:
                                # normalize by the denominator (row 64):
                                # DVE only does the fast reciprocal; ACT
                                # evacuates O; Pool broadcasts + multiplies
                                # (keeps the busy DVE queue unblocked)
                                ps_o = ps_os.pop(j)
                                den = stpool.tile([1, 512], F32, tag="den", name="den")
                                nc.vector.tensor_copy(den[:], ps_o[DH:DH + 1, :])
                                rec = stpool.tile([1, 512], F32, tag="rec", name="rec")
                                nc.vector.reciprocal_approx_fast(rec[:], den[:])
                                ot = stpool.tile([DH, 512], F32, tag="ot", bufs=2, name="ot")
                                nc.scalar.activation(ot[:], ps_o[:DH, :], AF.Copy)
                                recb = stpool.tile(
                                    [DH, 512], F32, tag="recb", bufs=2, name="recb"
                                )
                                nc.gpsimd.partition_broadcast(
                                    recb[:], rec[:], channels=DH
                                )
                                nc.vector.tensor_mul(
                                    ostack[h * DH:(h + 1) * DH, j * 512:(j + 1) * 512],
                                    ot[:], recb[:],
                                )
                        return go

                    # j=0: 4 tiles (all diagonal band); j=1: 8 tiles.
                    # Score tiles run now; each A@v chases 2 tiles behind,
                    # the tail spills into `pending` for the next phase.
                    run_q = []
                    for (i, j) in [(0, 0), (1, 0), (2, 0), (3, 0)] + [
                        (i, 1) for i in range(NT)
                    ]:
                        nm = 4 if j == 0 else NT
                        pt_cell = [None]
                        sc = mk_score(i, j)

                        def run_sc(sc=sc, pt_cell=pt_cell):
                            pt_cell[0] = sc()

                        def get_pt(pt_cell=pt_cell):
                            return pt_cell[0]

                        run_sc()
                        run_q.append(mk_av(i, j, nm, get_pt))
                        if len(run_q) > 2:
                            run_q.pop(0)()
                    pending.extend(run_q)

                # previous batch's output projection, interleaved into the
                # next batch's t-proj phase via `pending`. For the last
                # batch: tiles 0-3 only need the j=0 normalizes, so they
                # jump ahead of the deferred j=1 A@v tail.
                ops = emit_outproj(b, ostack)
                if b == B - 1:
                    pending[0:0] = ops[:8]
                    pending.extend(ops[8:])
                else:
                    pending.extend(ops)

            for fn in pending:
                fn()
    nc.finalize()
    return nc


def _get_nc():
    if "nc" not in _CACHE:
        _CACHE["nc"] = _build_nc()
    return _CACHE["nc"]


def make_in_maps(x, Wkqv, bkqv, Wp):
    bf16 = ml_dtypes.bfloat16
    f8 = ml_dtypes.float8_e4m3
    xt = np.ascontiguousarray(np.transpose(x, (0, 2, 1)))
    xt_b = xt.astype(bf16)
    xt_8 = xt.astype(f8)
    pidx = np.arange(P)[:, None]
    fidx = np.arange(512)[None, :]
    masks = np.stack(
        [(pidx + P * i <= fidx) for i in range(4)]
    ).astype(bf16)
    in_maps = []
    for c in range(NCORES):
        w_dt = f8 if MODE >= 3 else bf16
        w_sc = MSC if MODE >= 3 else 1.0
        wqT = np.stack(
            [
                np.ascontiguousarray(Wkqv[HL * c + hh, :, D:2 * D].T) * w_sc
                for hh in range(HL)
            ]
        ).astype(w_dt)
        wkT = np.stack(
            [
                np.ascontiguousarray(Wkqv[HL * c + hh, :, :D].T) * w_sc
                for hh in range(HL)
            ]
        ).astype(w_dt)
        wv = np.ascontiguousarray(
            np.concatenate(
                [Wkqv[HL * c + hh, :, 2 * D:] for hh in range(HL)], axis=1
            )
        ).astype(bf16)
        wp = np.ascontiguousarray(Wp[P * c:P * (c + 1)]).astype(bf16)
        in_maps.append({
            "xt": xt_b, "xt8": xt_8, "wqT": wqT, "wkT": wkT, "wv": wv,
            "wp": wp, "masks": masks,
        })
    return in_maps


def run(x, Wkqv, bkqv, Wp, bp, trace=False):
    nc = _get_nc()
    in_maps = make_in_maps(x, Wkqv, bkqv, Wp)
    res = run_bass_kernel_spmd(nc, in_maps, core_ids=list(range(NCORES)), trace=trace)
    total = None
    for r in res.results:
        part = r["out"].astype(np.float64)
        total = part if total is None else total + part
    out = (total + np.asarray(bp, np.float64)).astype(np.float32)
    return out, res


def kernel(x, Wkqv, bkqv, Wp, bp):
    out, _ = run(x, Wkqv, bkqv, Wp, bp, trace=False)
    return out


# revision 30
# speedup vs baseline: 1.0021x; 1.0021x over previous
"""Causal self-attention (per-head full-D k/q, DH-wide v) on 8 trn2 cores.

Sharding: tensor-parallel over heads. Core c owns heads (2c, 2c+1).

Key algebraic restructure vs the straightforward kernel: k and q are only
ever used through q @ k^T, so per head the two projections fold into one
matrix  M = Wq @ Wk^T  (computed once on device) and

  scores^T = x @ (x @ M)^T / sqrt(D)

replaces the 2*D-wide k/q projection (per batch) with a D-wide t = x@M
projection plus a once-per-head M precompute. All three matmuls in that
chain (M precompute, t = x@M, scores = x@t^T) run in fp8e4 with DoubleRow
perf mode (2 contraction tiles per instruction, ~2x PE rate): weights and
M are scaled by 64 so their values sit in e4m3's normal range; the exp()
activation scale folds the 1/64 back out along with the 1/sqrt(D).
Measured scale-relative absmax error 1.67e-2 (gate 2e-2); the all-bf16
fallback (MODE=0) measures 4.1e-3.

Per core, for all 4 batches:
  M[h]    = Wq8[h] @ Wk8[h]^T          (fp8 DR, f32 psum -> fp8*64)
  t^T     = M-contraction with x8^T    (fp8 DR, f32 psum -> fp8)
  S^T     = x8 @ t8^T                  (fp8 DR, 512-wide n blocks; causal:
                                        12 of 16 tiles, diagonal tiles
                                        column-trimmed to the valid range)
  P^T     = exp(S^T / 2048) * mask     (ACT exp -> bf16; 8 masked tiles)
  O^T_aug = [v | 1]^T @ P^T            (bf16; row 64 = softmax denominator)
  ostack  = O^T / den                  (DVE fast-reciprocal on the 1-row
                                        den, gpsimd partition_broadcast,
                                        DVE mul; ACT evacuates O)
  partial = ostack.T @ Wp[128c:+128]   (bf16; partials DMA'd out in bf16)
Host sums the 8 partials and adds bp.

v is computed directly in [tokens, v-col] orientation (lhsT = x tiles) so
no transpose is needed; both heads' 64 v columns per 128-wide matmul.

Scheduling: 16 warmup matmuls ramp the PE clock during the DMA lead-in;
M[h0], b0's v-projection, and M[h1] interleave with the staging DMAs; the
next batch's x is prefetched a full batch ahead; deferred A@v tails and
the previous batch's output projection drain one-per-tile inside the next
t-projection phase so the PE never waits on the exp (ACT) or normalize
(DVE/Pool) chains. PSUM evacuations alternate DVE/ACT (gpsimd has no
PSUM port and keeps a single ucode library loaded: partition_broadcast).

NOTE: bkqv is all-zeros per the problem spec and is not applied (the k/q
part would need rank-1 score corrections, the v part a per-column add).
"""

import sys
import types

import numpy as np
import ml_dtypes

import concourse.bass as bass
import concourse.bacc as bacc
import concourse.tile as tile
from concourse import mybir
from concourse.bass_utils import run_bass_kernel_spmd

# If BASS_TRACE is set in the environment, run_bass_kernel_spmd imports
# antenv.axon_hooks, which this image may not ship. Register a stub that
# reports "no hook" so tracing degrades gracefully instead of crashing.
try:
    from antenv.axon_hooks import get_axon_ntff_profile_hook  # noqa: F401
except ImportError:
    import antenv

    _mod = types.ModuleType("antenv.axon_hooks")
    _mod.get_axon_ntff_profile_hook = lambda: None
    _mod.set_axon_ntff_profile_hook = lambda h: setattr(
        _mod, "get_axon_ntff_profile_hook", lambda: h
    )
    antenv.axon_hooks = _mod
    sys.modules["antenv.axon_hooks"] = _mod

BF16 = mybir.dt.bfloat16
F32 = mybir.dt.float32
F8 = mybir.dt.float8e4
AF = mybir.ActivationFunctionType
DR = mybir.MatmulPerfMode.DoubleRow

B, N, D, H, DH = 4, 1024, 1024, 16, 64
P = 128
NCORES = 8
HL = H // NCORES        # 2 local heads per core
DT = D // P             # 8 contraction tiles
DP = DT // 2            # 4 DoubleRow contraction pairs
NJ = N // 512           # 2 512-wide n blocks
NT = N // P             # 8 token tiles
VG = 96                 # v group stride ([v(64) | ones(1)] padded)
MSC = 64.0              # fp8 scale baked into M (and t, scores)
EXPSCALE = 1.0 / (32.0 * MSC)

# numeric mode: 3 = also fp8 M-precompute, 2 = fp8 t-proj + fp8 scores
# (DoubleRow), 1 = fp8 t-proj only, 0 = all-bf16 (structure validation)
MODE = 3

_CACHE = {}


def _build_nc():
    nc = bacc.Bacc(
        "TRN2",
        target_bir_lowering=False,
        debug=False,
        enable_asserts=True,
        num_devices=NCORES,
    )
    xt_d = nc.declare_dram_parameter("xt", [B, D, N], BF16, isOutput=False)
    x8_d = nc.declare_dram_parameter("xt8", [B, D, N], F8, isOutput=False)
    W_DT = F8 if MODE >= 3 else BF16
    wq_d = nc.declare_dram_parameter("wqT", [HL, D, D], W_DT, isOutput=False)
    wk_d = nc.declare_dram_parameter("wkT", [HL, D, D], W_DT, isOutput=False)
    wv_d = nc.declare_dram_parameter("wv", [D, HL * DH], BF16, isOutput=False)
    wp_d = nc.declare_dram_parameter("wp", [P, D], BF16, isOutput=False)
    mask_d = nc.declare_dram_parameter("masks", [4, P, 512], BF16, isOutput=False)
    out_d = nc.declare_dram_parameter("out", [B, N, D], BF16, isOutput=True)

    with tile.TileContext(nc) as tc:
        with (
            tc.tile_pool(name="const", bufs=1) as constp,
            tc.tile_pool(name="mpool", bufs=1) as mpool,
            tc.tile_pool(name="stagep", bufs=(2 if MODE >= 1 else 1)) as stagep,
            tc.tile_pool(name="xpool", bufs=2) as xpool,
            tc.tile_pool(name="x8pool", bufs=2) as x8pool,
            tc.tile_pool(name="ttpool", bufs=2) as ttpool,
            tc.tile_pool(name="vpool", bufs=2) as vpool,
            tc.tile_pool(name="ptpool", bufs=10) as ptpool,
            tc.tile_pool(name="otpool", bufs=2) as otpool,
            tc.tile_pool(name="stpool", bufs=3) as stpool,
            tc.tile_pool(name="pspool", bufs=1, space="PSUM") as pspool,
        ):
            # ---- resident constants ----
            wv_sb = constp.tile([P, DT * HL * DH], BF16, name="wv_sb")
            wp_sb = constp.tile([P, D], BF16, name="wp_sb")
            mask_sb = constp.tile([P, 4 * 512], BF16, name="mask_sb")
            # per-head M scaled by MSC: [d-in-tile, d-tile, d-out]
            M_DT = F8 if MODE >= 1 else BF16
            TT_DT = F8 if MODE >= 2 else BF16
            m8 = [
                mpool.tile([P, DT, D], M_DT, name=f"m8_{h}") for h in range(HL)
            ]

            def emit_vproj(xt_sb):
                # packed v projection, computed directly in [tokens, v-cols]
                # orientation (lhsT = x tiles), so no transpose is needed
                v_sb = vpool.tile([P, HL * NT * VG], BF16, tag="vaug", name="v_sb")
                nc.vector.memset(v_sb[:, :], 1.0)
                for i in range(NT):
                    ps_v = pspool.tile([P, P], F32, tag="ps", bufs=4, name="ps_v")
                    for d in range(DT):
                        nc.tensor.matmul(
                            ps_v[:],
                            lhsT=xt_sb[:, d, i * P:(i + 1) * P],
                            rhs=wv_sb[:, d * P:(d + 1) * P],
                            start=(d == 0),
                            stop=(d == DT - 1),
                        )
                    for hh in range(HL):
                        dst = v_sb[:, (hh * NT + i) * VG:(hh * NT + i) * VG + DH]
                        if (i + hh) % 2 == 0:
                            nc.vector.tensor_copy(dst, ps_v[:, hh * DH:(hh + 1) * DH])
                        else:
                            nc.scalar.activation(dst, ps_v[:, hh * DH:(hh + 1) * DH], AF.Copy)
                return v_sb

            def fetch_x(b, xt_sb=None, x8_sb=None):
                if xt_sb is None:
                    xt_sb = xpool.tile([P, DT, N], BF16, tag="xt", name="xt_sb")
                    for d in range(DT):
                        nc.sync.dma_start(
                            out=xt_sb[:, d, :], in_=xt_d[b, d * P:(d + 1) * P, :]
                        )
                if x8_sb is None:
                    x8_sb = x8pool.tile([P, DT, N], F8, tag="x8", name="x8_sb")
                    for d in range(DT):
                        nc.sync.dma_start(
                            out=x8_sb[:, d, :], in_=x8_d[b, d * P:(d + 1) * P, :]
                        )
                return xt_sb, x8_sb

            # ---- PE warmup: ramp the clock while the first DMAs land ----
            warm = constp.tile([P, 512], BF16, name="warm")
            nc.vector.memset(warm[:, :], 0.0)
            for k in range(16):
                ps_w = pspool.tile([P, 512], F32, tag="ps", bufs=4, name="ps_w")
                nc.tensor.matmul(
                    ps_w[:], lhsT=warm[:, 0:P], rhs=warm[:], start=True, stop=True
                )

            # ---- M precompute: per head, M = Wq @ Wk^T via transposed
            # weight tiles (contraction over e on partitions). b0's v
            # projection runs between the two heads, covering head 1's
            # staging DMA. ----
            for h in range(HL):
                wqs = stagep.tile([P, DT, D], W_DT, tag="wqs", name="wqs")
                wks = stagep.tile([P, DT, D], W_DT, tag="wks", name="wks")
                for e in range(DT):
                    nc.sync.dma_start(
                        out=wqs[:, e, :], in_=wq_d[h, e * P:(e + 1) * P, :]
                    )
                    nc.sync.dma_start(
                        out=wks[:, e, :], in_=wk_d[h, e * P:(e + 1) * P, :]
                    )
                if h == 0:
                    # b0's v-projection inputs, right behind h0's staging
                    for dd in range(DT):
                        nc.sync.dma_start(
                            out=wv_sb[:, dd * P:(dd + 1) * P],
                            in_=wv_d[dd * P:(dd + 1) * P, :],
                        )
                    xt_sb0, _ = fetch_x(0, x8_sb="later")
                else:
                    _, x8_sb0 = fetch_x(0, xt_sb="later")
                    for m in range(4):
                        nc.sync.dma_start(
                            out=mask_sb[:, m * 512:(m + 1) * 512], in_=mask_d[m]
                        )
                    nc.sync.dma_start(out=wp_sb[:], in_=wp_d[:])
                for dd in range(DT):
                    for jm in range(NJ):
                        ps_m = pspool.tile([P, 512], F32, tag="ps", bufs=4, name="ps_m")
                        if MODE >= 3:
                            # host pre-scales Wq/Wk by 64 -> psum holds
                            # 4096*M; rescale to MSC*M on evacuation
                            for ep in range(DP):
                                nc.tensor.matmul(
                                    ps_m[:],
                                    lhsT=wqs[:, 2 * ep:2 * ep + 2, dd * P:(dd + 1) * P],
                                    rhs=wks[:, 2 * ep:2 * ep + 2, jm * 512:(jm + 1) * 512],
                                    start=(ep == 0),
                                    stop=(ep == DP - 1),
                                    perf_mode=DR,
                                )
                        else:
                            for e in range(DT):
                                nc.tensor.matmul(
                                    ps_m[:],
                                    lhsT=wqs[:, e, dd * P:(dd + 1) * P],
                                    rhs=wks[:, e, jm * 512:(jm + 1) * 512],
                                    start=(e == 0),
                                    stop=(e == DT - 1),
                                )
                        nc.scalar.activation(
                            m8[h][:, dd, jm * 512:(jm + 1) * 512],
                            ps_m[:],
                            AF.Copy,
                            scale=(MSC / (MSC * MSC) if MODE >= 3 else MSC),
                        )
                if h == 0:
                    # b0's v projection covers head 1's staging DMA wait
                    v_sb0 = emit_vproj(xt_sb0)

            # ---- deferred emissions keep the PE fed ----
            pending = []  # list of closures, interleaved into t-proj phases

            def emit_outproj(bb, ost):
                insts = []
                for t in range(NT):
                    for j2 in range(NJ):
                        def mk(t=t, j2=j2, bb=bb, ost=ost):
                            ps_f = pspool.tile(
                                [P, 512], F32, tag="psf", bufs=2, name="ps_f"
                            )
                            nc.tensor.matmul(
                                ps_f[:],
                                lhsT=ost[:, t * P:(t + 1) * P],
                                rhs=wp_sb[:, j2 * 512:(j2 + 1) * 512],
                                start=True, stop=True,
                            )
                            stage = stpool.tile(
                                [P, 512], BF16, tag="stage", name="stage"
                            )
                            if (t + j2) % 2 == 0:
                                nc.scalar.activation(stage[:], ps_f[:], AF.Copy)
                            else:
                                nc.vector.tensor_copy(stage[:], ps_f[:])
                            nc.sync.dma_start(
                                out=out_d[
                                    bb, t * P:(t + 1) * P,
                                    j2 * 512:(j2 + 1) * 512,
                                ],
                                in_=stage[:],
                            )
                        insts.append(mk)
                return insts

            def drain_pending(k):
                for _ in range(k):
                    if pending:
                        pending.pop(0)()

            next_x = (xt_sb0, x8_sb0)
            for b in range(B):
                xt_sb, x8_sb = next_x
                if b + 1 < B:
                    next_x = fetch_x(b + 1)

                if b == 0:
                    v_sb = v_sb0  # emitted during the M-precompute phase
                else:
                    v_sb = emit_vproj(xt_sb)

                ostack = otpool.tile([P, N], BF16, tag="ostack", name="ostack")

                for h in range(HL):
                    # ---- t projection: t^T[d'-tile, n] ----
                    tt8 = ttpool.tile([P, DT, N], TT_DT, tag="tt", name="tt8")
                    nmm = 0
                    for jh in range(NJ):
                        for dd in range(DT):
                            ps_y = pspool.tile(
                                [P, 512], F32, tag="ps", bufs=4, name="ps_y"
                            )
                            if MODE >= 1:
                                for dp in range(DP):
                                    nc.tensor.matmul(
                                        ps_y[:],
                                        lhsT=m8[h][:, 2 * dp:2 * dp + 2, dd * P:(dd + 1) * P],
                                        rhs=x8_sb[:, 2 * dp:2 * dp + 2, jh * 512:(jh + 1) * 512],
                                        start=(dp == 0),
                                        stop=(dp == DP - 1),
                                        perf_mode=DR,
                                    )
                            else:
                                for d in range(DT):
                                    nc.tensor.matmul(
                                        ps_y[:],
                                        lhsT=m8[h][:, d, dd * P:(dd + 1) * P],
                                        rhs=xt_sb[:, d, jh * 512:(jh + 1) * 512],
                                        start=(d == 0),
                                        stop=(d == DT - 1),
                                    )
                            # evacuate psum alternating DVE/ACT (gpsimd has
                            # no PSUM port)
                            if dd % 2 == 0:
                                nc.vector.tensor_copy(
                                    tt8[:, dd, jh * 512:(jh + 1) * 512], ps_y[:]
                                )
                            else:
                                nc.scalar.activation(
                                    tt8[:, dd, jh * 512:(jh + 1) * 512],
                                    ps_y[:], AF.Copy,
                                )
                            nmm += 1
                            if nmm >= 5:
                                drain_pending(1)

                    # ---- scores + A@v per 512-wide n block ----
                    ps_os = {}

                    def mk_score(i, j, h=h, tt8=tt8, x8_sb=x8_sb, xt_sb=xt_sb):
                        def go():
                            # diagonal tiles: the first mi*128 columns are
                            # fully masked -- skip computing them entirely
                            # (A@v consumes the matching sub-range only)
                            mi = i - 4 * j
                            c0 = mi * P if mi > 0 else 0
                            ps_s = pspool.tile(
                                [P, 512], F32, tag="ps", bufs=4, name="ps_s"
                            )
                            if MODE >= 2:
                                for dp in range(DP):
                                    nc.tensor.matmul(
                                        ps_s[:, c0:],
                                        lhsT=x8_sb[:, 2 * dp:2 * dp + 2, i * P:(i + 1) * P],
                                        rhs=tt8[:, 2 * dp:2 * dp + 2, j * 512 + c0:(j + 1) * 512],
                                        start=(dp == 0),
                                        stop=(dp == DP - 1),
                                        perf_mode=DR,
                                    )
                            else:
                                for d in range(DT):
                                    nc.tensor.matmul(
                                        ps_s[:, c0:],
                                        lhsT=xt_sb[:, d, i * P:(i + 1) * P],
                                        rhs=tt8[:, d, j * 512 + c0:(j + 1) * 512],
                                        start=(d == 0),
                                        stop=(d == DT - 1),
                                    )
                            pt = ptpool.tile([P, 512], BF16, tag="pt", name="pt")
                            nc.scalar.activation(
                                pt[:, c0:], ps_s[:, c0:], AF.Exp, scale=EXPSCALE
                            )
                            if mi >= 0:  # diagonal-crossing tile
                                pt2 = ptpool.tile(
                                    [P, 512], BF16, tag="pt2", bufs=6, name="pt2"
                                )
                                nc.vector.tensor_mul(
                                    pt2[:, c0:], pt[:, c0:],
                                    mask_sb[:, mi * 512 + c0:(mi + 1) * 512],
                                )
                                return pt2, c0
                            return pt, c0
                        return go

                    def mk_av(i, j, nm, pt_get, h=h, b=b, ostack=ostack, v_sb=v_sb):
                        def go():
                            pt, c0 = pt_get()
                            if j not in ps_os:
                                ps_os[j] = pspool.tile(
                                    [DH + 1, 512], F32, tag="po", bufs=2, name="ps_o"
                                )
                            nc.tensor.matmul(
                                ps_os[j][:, c0:],
                                lhsT=v_sb[:, (h * NT + i) * VG:(h * NT + i) * VG + DH + 1],
                                rhs=pt[:, c0:],
                                start=(i == 0),
                                stop=(i == nm - 1),
                            )
                            if i == nm - 1:
                                # normalize by the denominator (row 64):
                                # DVE only does the fast reciprocal; ACT
                                # evacuates O; Pool broadcasts + multiplies
                                # (keeps the busy DVE queue unblocked)
                                ps_o = ps_os.pop(j)
                                den = stpool.tile([1, 512], F32, tag="den", name="den")
                                nc.vector.tensor_copy(den[:], ps_o[DH:DH + 1, :])
                                rec = stpool.tile([1, 512], F32, tag="rec", name="rec")
                                nc.vector.reciprocal_approx_fast(rec[:], den[:])
                                ot = stpool.tile([DH, 512], F32, tag="ot", bufs=2, name="ot")
                                nc.scalar.activation(ot[:], ps_o[:DH, :], AF.Copy)
                                recb = stpool.tile(
                                    [DH, 512], F32, tag="recb", bufs=2, name="recb"
                                )
                                nc.gpsimd.partition_broadcast(
                                    recb[:], rec[:], channels=DH
                                )
                                nc.vector.tensor_mul(
                                    ostack[h * DH:(h + 1) * DH, j * 512:(j + 1) * 512],
                                    ot[:], recb[:],
                                )
                        return go

                    # j=0: 4 tiles (all diagonal band); j=1: 8 tiles.
                    # Score tiles run now; each A@v chases 2 tiles behind,
                    # the tail spills into `pending` for the next phase.
                    # Last batch, last head: the output-projection tiles
                    # that only need the j=0 normalizes (emitted by
                    # iteration 5 inside the A@v chase) drain inside this
                    # loop so the endgame chain stays overlapped.
                    last = (b == B - 1 and h == HL - 1)
                    b3ops = emit_outproj(b, ostack) if last else None
                    b3k = 0
                    run_q = []
                    for idx, (i, j) in enumerate(
                        [(0, 0), (1, 0), (2, 0), (3, 0)]
                        + [(i, 1) for i in range(NT)]
                    ):
                        nm = 4 if j == 0 else NT
                        pt_cell = [None]
                        sc = mk_score(i, j)

                        def run_sc(sc=sc, pt_cell=pt_cell):
                            pt_cell[0] = sc()

                        def get_pt(pt_cell=pt_cell):
                            return pt_cell[0]

                        run_sc()
                        run_q.append(mk_av(i, j, nm, get_pt))
                        if len(run_q) > 2:
                            run_q.pop(0)()
                        if b3ops is not None and idx >= 6:
                            for _ in range(2):
                                if b3k < 8:
                                    b3ops[b3k]()
                                    b3k += 1
                    pending.extend(run_q)
                    if b3ops is not None:
                        pending.extend(b3ops[8:])

                # previous batch's output projection, interleaved into the
                # next batch's t-proj phase via `pending`. For the last
                # batch: tiles 0-3 only need the j=0 normalizes, so they
                # jump ahead of the deferred j=1 A@v tail.
                if b < B - 1:
                    pending.extend(emit_outproj(b, ostack))

            for fn in pending:
                fn()
    nc.finalize()
    return nc


def _get_nc():
    if "nc" not in _CACHE:
        _CACHE["nc"] = _build_nc()
    return _CACHE["nc"]


def make_in_maps(x, Wkqv, bkqv, Wp):
    bf16 = ml_dtypes.bfloat16
    f8 = ml_dtypes.float8_e4m3
    xt = np.ascontiguousarray(np.transpose(x, (0, 2, 1)))
    xt_b = xt.astype(bf16)
    xt_8 = xt.astype(f8)
    pidx = np.arange(P)[:, None]
    fidx = np.arange(512)[None, :]
    masks = np.stack(
        [(pidx + P * i <= fidx) for i in range(4)]
    ).astype(bf16)
    in_maps = []
    for c in range(NCORES):
        w_dt = f8 if MODE >= 3 else bf16
        w_sc = MSC if MODE >= 3 else 1.0
        wqT = np.stack(
            [
                np.ascontiguousarray(Wkqv[HL * c + hh, :, D:2 * D].T) * w_sc
                for hh in range(HL)
            ]
        ).astype(w_dt)
        wkT = np.stack(
            [
                np.ascontiguousarray(Wkqv[HL * c + hh, :, :D].T) * w_sc
                for hh in range(HL)
            ]
        ).astype(w_dt)
        wv = np.ascontiguousarray(
            np.concatenate(
                [Wkqv[HL * c + hh, :, 2 * D:] for hh in range(HL)], axis=1
            )
        ).astype(bf16)
        wp = np.ascontiguousarray(Wp[P * c:P * (c + 1)]).astype(bf16)
        in_maps.append({
            "xt": xt_b, "xt8": xt_8, "wqT": wqT, "wkT": wkT, "wv": wv,
            "wp": wp, "masks": masks,
        })
    return in_maps


def run(x, Wkqv, bkqv, Wp, bp, trace=False):
    nc = _get_nc()
    in_maps = make_in_maps(x, Wkqv, bkqv, Wp)
    res = run_bass_kernel_spmd(nc, in_maps, core_ids=list(range(NCORES)), trace=trace)
    total = None
    for r in res.results:
        part = r["out"].astype(np.float64)
        total = part if total is None else total + part
    out = (total + np.asarray(bp, np.float64)).astype(np.float32)
    return out, res


def kernel(x, Wkqv, bkqv, Wp, bp):
    out, _ = run(x, Wkqv, bkqv, Wp, bp, trace=False)
    return out


# revision 31
# speedup vs baseline: 1.0048x; 1.0027x over previous
"""Causal self-attention (per-head full-D k/q, DH-wide v) on 8 trn2 cores.

Sharding: tensor-parallel over heads. Core c owns heads (2c, 2c+1).

Key algebraic restructure vs the straightforward kernel: k and q are only
ever used through q @ k^T, so per head the two projections fold into one
matrix  M = Wq @ Wk^T  (computed once on device) and

  scores^T = x @ (x @ M)^T / sqrt(D)

replaces the 2*D-wide k/q projection (per batch) with a D-wide t = x@M
projection plus a once-per-head M precompute. All three matmuls in that
chain (M precompute, t = x@M, scores = x@t^T) run in fp8e4 with DoubleRow
perf mode (2 contraction tiles per instruction, ~2x PE rate): weights and
M are scaled by 64 so their values sit in e4m3's normal range; the exp()
activation scale folds the 1/64 back out along with the 1/sqrt(D).
Measured scale-relative absmax error 1.67e-2 (gate 2e-2); the all-bf16
fallback (MODE=0) measures 4.1e-3.

Per core, for all 4 batches:
  M[h]    = Wq8[h] @ Wk8[h]^T          (fp8 DR, f32 psum -> fp8*64)
  t^T     = M-contraction with x8^T    (fp8 DR, f32 psum -> fp8)
  S^T     = x8 @ t8^T                  (fp8 DR, 512-wide n blocks; causal:
                                        12 of 16 tiles, diagonal tiles
                                        column-trimmed to the valid range)
  P^T     = exp(S^T / 2048) * mask     (ACT exp -> bf16; 8 masked tiles)
  O^T_aug = [v | 1]^T @ P^T            (bf16; row 64 = softmax denominator)
  ostack  = O^T / den                  (DVE fast-reciprocal on the 1-row
                                        den, gpsimd partition_broadcast,
                                        DVE mul; ACT evacuates O)
  partial = ostack.T @ Wp[128c:+128]   (bf16; partials DMA'd out in bf16)
Host sums the 8 partials and adds bp.

v is computed directly in [tokens, v-col] orientation (lhsT = x tiles) so
no transpose is needed; both heads' 64 v columns per 128-wide matmul.

Scheduling: 16 warmup matmuls ramp the PE clock during the DMA lead-in;
M[h0], b0's v-projection, and M[h1] interleave with the staging DMAs; the
next batch's x is prefetched a full batch ahead; deferred A@v tails and
the previous batch's output projection drain one-per-tile inside the next
t-projection phase so the PE never waits on the exp (ACT) or normalize
(DVE/Pool) chains. PSUM evacuations alternate DVE/ACT (gpsimd has no
PSUM port and keeps a single ucode library loaded: partition_broadcast).

NOTE: bkqv is all-zeros per the problem spec and is not applied (the k/q
part would need rank-1 score corrections, the v part a per-column add).
"""

import sys
import types

import numpy as np
import ml_dtypes

import concourse.bass as bass
import concourse.bacc as bacc
import concourse.tile as tile
from concourse import mybir
from concourse.bass_utils import run_bass_kernel_spmd

# If BASS_TRACE is set in the environment, run_bass_kernel_spmd imports
# antenv.axon_hooks, which this image may not ship. Register a stub that
# reports "no hook" so tracing degrades gracefully instead of crashing.
try:
    from antenv.axon_hooks import get_axon_ntff_profile_hook  # noqa: F401
except ImportError:
    import antenv

    _mod = types.ModuleType("antenv.axon_hooks")
    _mod.get_axon_ntff_profile_hook = lambda: None
    _mod.set_axon_ntff_profile_hook = lambda h: setattr(
        _mod, "get_axon_ntff_profile_hook", lambda: h
    )
    antenv.axon_hooks = _mod
    sys.modules["antenv.axon_hooks"] = _mod

BF16 = mybir.dt.bfloat16
F32 = mybir.dt.float32
F8 = mybir.dt.float8e4
AF = mybir.ActivationFunctionType
DR = mybir.MatmulPerfMode.DoubleRow

B, N, D, H, DH = 4, 1024, 1024, 16, 64
P = 128
NCORES = 8
HL = H // NCORES        # 2 local heads per core
DT = D // P             # 8 contraction tiles
DP = DT // 2            # 4 DoubleRow contraction pairs
NJ = N // 512           # 2 512-wide n blocks
NT = N // P             # 8 token tiles
VG = 96                 # v group stride ([v(64) | ones(1)] padded)
MSC = 64.0              # fp8 scale baked into M (and t, scores)
EXPSCALE = 1.0 / (32.0 * MSC)

# numeric mode: 3 = also fp8 M-precompute, 2 = fp8 t-proj + fp8 scores
# (DoubleRow), 1 = fp8 t-proj only, 0 = all-bf16 (structure validation)
MODE = 3

_CACHE = {}


def _build_nc():
    nc = bacc.Bacc(
        "TRN2",
        target_bir_lowering=False,
        debug=False,
        enable_asserts=True,
        num_devices=NCORES,
    )
    xt_d = nc.declare_dram_parameter("xt", [B, D, N], BF16, isOutput=False)
    x8_d = nc.declare_dram_parameter("xt8", [B, D, N], F8, isOutput=False)
    W_DT = F8 if MODE >= 3 else BF16
    wq_d = nc.declare_dram_parameter("wqT", [HL, D, D], W_DT, isOutput=False)
    wk_d = nc.declare_dram_parameter("wkT", [HL, D, D], W_DT, isOutput=False)
    wv_d = nc.declare_dram_parameter("wv", [D, HL * DH], BF16, isOutput=False)
    wp_d = nc.declare_dram_parameter("wp", [P, D], BF16, isOutput=False)
    mask_d = nc.declare_dram_parameter("masks", [4, P, 512], BF16, isOutput=False)
    out_d = nc.declare_dram_parameter("out", [B, N, D], BF16, isOutput=True)

    with tile.TileContext(nc) as tc:
        with (
            tc.tile_pool(name="const", bufs=1) as constp,
            tc.tile_pool(name="mpool", bufs=1) as mpool,
            tc.tile_pool(name="stagep", bufs=(2 if MODE >= 1 else 1)) as stagep,
            tc.tile_pool(name="xpool", bufs=2) as xpool,
            tc.tile_pool(name="x8pool", bufs=2) as x8pool,
            tc.tile_pool(name="ttpool", bufs=2) as ttpool,
            tc.tile_pool(name="vpool", bufs=2) as vpool,
            tc.tile_pool(name="ptpool", bufs=10) as ptpool,
            tc.tile_pool(name="otpool", bufs=2) as otpool,
            tc.tile_pool(name="stpool", bufs=3) as stpool,
            tc.tile_pool(name="pspool", bufs=1, space="PSUM") as pspool,
        ):
            # ---- resident constants ----
            wv_sb = constp.tile([P, DT * HL * DH], BF16, name="wv_sb")
            wp_sb = constp.tile([P, D], BF16, name="wp_sb")
            mask_sb = constp.tile([P, 4 * 512], BF16, name="mask_sb")
            # per-head M scaled by MSC: [d-in-tile, d-tile, d-out]
            M_DT = F8 if MODE >= 1 else BF16
            TT_DT = F8 if MODE >= 2 else BF16
            m8 = [
                mpool.tile([P, DT, D], M_DT, name=f"m8_{h}") for h in range(HL)
            ]

            def emit_vproj(xt_sb):
                # packed v projection, computed directly in [tokens, v-cols]
                # orientation (lhsT = x tiles), so no transpose is needed
                v_sb = vpool.tile([P, HL * NT * VG], BF16, tag="vaug", name="v_sb")
                nc.vector.memset(v_sb[:, :], 1.0)
                for i in range(NT):
                    ps_v = pspool.tile([P, P], F32, tag="ps", bufs=4, name="ps_v")
                    for d in range(DT):
                        nc.tensor.matmul(
                            ps_v[:],
                            lhsT=xt_sb[:, d, i * P:(i + 1) * P],
                            rhs=wv_sb[:, d * P:(d + 1) * P],
                            start=(d == 0),
                            stop=(d == DT - 1),
                        )
                    for hh in range(HL):
                        dst = v_sb[:, (hh * NT + i) * VG:(hh * NT + i) * VG + DH]
                        if (i + hh) % 2 == 0:
                            nc.vector.tensor_copy(dst, ps_v[:, hh * DH:(hh + 1) * DH])
                        else:
                            nc.scalar.activation(dst, ps_v[:, hh * DH:(hh + 1) * DH], AF.Copy)
                return v_sb

            def fetch_x(b, xt_sb=None, x8_sb=None):
                if xt_sb is None:
                    xt_sb = xpool.tile([P, DT, N], BF16, tag="xt", name="xt_sb")
                    for d in range(DT):
                        nc.sync.dma_start(
                            out=xt_sb[:, d, :], in_=xt_d[b, d * P:(d + 1) * P, :]
                        )
                if x8_sb is None:
                    x8_sb = x8pool.tile([P, DT, N], F8, tag="x8", name="x8_sb")
                    for d in range(DT):
                        nc.sync.dma_start(
                            out=x8_sb[:, d, :], in_=x8_d[b, d * P:(d + 1) * P, :]
                        )
                return xt_sb, x8_sb

            # ---- PE warmup: ramp the clock while the first DMAs land ----
            warm = constp.tile([P, 512], BF16, name="warm")
            nc.vector.memset(warm[:, :], 0.0)
            for k in range(16):
                ps_w = pspool.tile([P, 512], F32, tag="ps", bufs=4, name="ps_w")
                nc.tensor.matmul(
                    ps_w[:], lhsT=warm[:, 0:P], rhs=warm[:], start=True, stop=True
                )

            # ---- M precompute: per head, M = Wq @ Wk^T via transposed
            # weight tiles (contraction over e on partitions). b0's v
            # projection runs between the two heads, covering head 1's
            # staging DMA. ----
            for h in range(HL):
                wqs = stagep.tile([P, DT, D], W_DT, tag="wqs", name="wqs")
                wks = stagep.tile([P, DT, D], W_DT, tag="wks", name="wks")
                for e in range(DT):
                    nc.sync.dma_start(
                        out=wqs[:, e, :], in_=wq_d[h, e * P:(e + 1) * P, :]
                    )
                    nc.sync.dma_start(
                        out=wks[:, e, :], in_=wk_d[h, e * P:(e + 1) * P, :]
                    )
                if h == 0:
                    # b0's v-projection inputs, right behind h0's staging
                    for dd in range(DT):
                        nc.sync.dma_start(
                            out=wv_sb[:, dd * P:(dd + 1) * P],
                            in_=wv_d[dd * P:(dd + 1) * P, :],
                        )
                    xt_sb0, _ = fetch_x(0, x8_sb="later")
                else:
                    _, x8_sb0 = fetch_x(0, xt_sb="later")
                    for m in range(4):
                        nc.sync.dma_start(
                            out=mask_sb[:, m * 512:(m + 1) * 512], in_=mask_d[m]
                        )
                    nc.sync.dma_start(out=wp_sb[:], in_=wp_d[:])
                for dd in range(DT):
                    for jm in range(NJ):
                        ps_m = pspool.tile([P, 512], F32, tag="ps", bufs=4, name="ps_m")
                        if MODE >= 3:
                            # host pre-scales Wq/Wk by 64 -> psum holds
                            # 4096*M; rescale to MSC*M on evacuation
                            for ep in range(DP):
                                nc.tensor.matmul(
                                    ps_m[:],
                                    lhsT=wqs[:, 2 * ep:2 * ep + 2, dd * P:(dd + 1) * P],
                                    rhs=wks[:, 2 * ep:2 * ep + 2, jm * 512:(jm + 1) * 512],
                                    start=(ep == 0),
                                    stop=(ep == DP - 1),
                                    perf_mode=DR,
                                )
                        else:
                            for e in range(DT):
                                nc.tensor.matmul(
                                    ps_m[:],
                                    lhsT=wqs[:, e, dd * P:(dd + 1) * P],
                                    rhs=wks[:, e, jm * 512:(jm + 1) * 512],
                                    start=(e == 0),
                                    stop=(e == DT - 1),
                                )
                        nc.scalar.activation(
                            m8[h][:, dd, jm * 512:(jm + 1) * 512],
                            ps_m[:],
                            AF.Copy,
                            scale=(MSC / (MSC * MSC) if MODE >= 3 else MSC),
                        )
                if h == 0:
                    # b0's v projection covers head 1's staging DMA wait
                    v_sb0 = emit_vproj(xt_sb0)

            # ---- deferred emissions keep the PE fed ----
            pending = []  # list of closures, interleaved into t-proj phases

            def emit_outproj(bb, ost):
                insts = []
                for t in range(NT):
                    for j2 in range(NJ):
                        def mk(t=t, j2=j2, bb=bb, ost=ost):
                            ps_f = pspool.tile(
                                [P, 512], F32, tag="psf", bufs=2, name="ps_f"
                            )
                            nc.tensor.matmul(
                                ps_f[:],
                                lhsT=ost[:, t * P:(t + 1) * P],
                                rhs=wp_sb[:, j2 * 512:(j2 + 1) * 512],
                                start=True, stop=True,
                            )
                            stage = stpool.tile(
                                [P, 512], BF16, tag="stage", name="stage"
                            )
                            if (t + j2) % 2 == 0:
                                nc.scalar.activation(stage[:], ps_f[:], AF.Copy)
                            else:
                                nc.vector.tensor_copy(stage[:], ps_f[:])
                            nc.sync.dma_start(
                                out=out_d[
                                    bb, t * P:(t + 1) * P,
                                    j2 * 512:(j2 + 1) * 512,
                                ],
                                in_=stage[:],
                            )
                        insts.append(mk)
                return insts

            def drain_pending(k):
                for _ in range(k):
                    if pending:
                        pending.pop(0)()

            next_x = (xt_sb0, x8_sb0)
            for b in range(B):
                xt_sb, x8_sb = next_x
                if b + 1 < B:
                    next_x = fetch_x(b + 1)

                if b == 0:
                    v_sb = v_sb0  # emitted during the M-precompute phase
                else:
                    v_sb = emit_vproj(xt_sb)

                ostack = otpool.tile([P, N], BF16, tag="ostack", name="ostack")

                for h in range(HL):
                    # ---- t projection: t^T[d'-tile, n] ----
                    tt8 = ttpool.tile([P, DT, N], TT_DT, tag="tt", name="tt8")
                    nmm = 0
                    for jh in range(NJ):
                        for dd in range(DT):
                            ps_y = pspool.tile(
                                [P, 512], F32, tag="ps", bufs=4, name="ps_y"
                            )
                            if MODE >= 1:
                                for dp in range(DP):
                                    nc.tensor.matmul(
                                        ps_y[:],
                                        lhsT=m8[h][:, 2 * dp:2 * dp + 2, dd * P:(dd + 1) * P],
                                        rhs=x8_sb[:, 2 * dp:2 * dp + 2, jh * 512:(jh + 1) * 512],
                                        start=(dp == 0),
                                        stop=(dp == DP - 1),
                                        perf_mode=DR,
                                    )
                            else:
                                for d in range(DT):
                                    nc.tensor.matmul(
                                        ps_y[:],
                                        lhsT=m8[h][:, d, dd * P:(dd + 1) * P],
                                        rhs=xt_sb[:, d, jh * 512:(jh + 1) * 512],
                                        start=(d == 0),
                                        stop=(d == DT - 1),
                                    )
                            # evacuate psum alternating DVE/ACT (gpsimd has
                            # no PSUM port)
                            if dd % 2 == 0:
                                nc.vector.tensor_copy(
                                    tt8[:, dd, jh * 512:(jh + 1) * 512], ps_y[:]
                                )
                            else:
                                nc.scalar.activation(
                                    tt8[:, dd, jh * 512:(jh + 1) * 512],
                                    ps_y[:], AF.Copy,
                                )
                            nmm += 1
                            if nmm >= 5:
                                drain_pending(1)

                    # ---- scores + A@v per 512-wide n block ----
                    ps_os = {}

                    def mk_score(i, j, h=h, tt8=tt8, x8_sb=x8_sb, xt_sb=xt_sb):
                        def go():
                            # diagonal tiles: the first mi*128 columns are
                            # fully masked -- skip computing them entirely
                            # (A@v consumes the matching sub-range only)
                            mi = i - 4 * j
                            c0 = mi * P if mi > 0 else 0
                            ps_s = pspool.tile(
                                [P, 512], F32, tag="ps", bufs=4, name="ps_s"
                            )
                            if MODE >= 2:
                                for dp in range(DP):
                                    nc.tensor.matmul(
                                        ps_s[:, c0:],
                                        lhsT=x8_sb[:, 2 * dp:2 * dp + 2, i * P:(i + 1) * P],
                                        rhs=tt8[:, 2 * dp:2 * dp + 2, j * 512 + c0:(j + 1) * 512],
                                        start=(dp == 0),
                                        stop=(dp == DP - 1),
                                        perf_mode=DR,
                                    )
                            else:
                                for d in range(DT):
                                    nc.tensor.matmul(
                                        ps_s[:, c0:],
                                        lhsT=xt_sb[:, d, i * P:(i + 1) * P],
                                        rhs=tt8[:, d, j * 512 + c0:(j + 1) * 512],
                                        start=(d == 0),
                                        stop=(d == DT - 1),
                                    )
                            pt = ptpool.tile([P, 512], BF16, tag="pt", name="pt")
                            nc.scalar.activation(
                                pt[:, c0:], ps_s[:, c0:], AF.Exp, scale=EXPSCALE
                            )
                            if mi >= 0:  # diagonal-crossing tile
                                pt2 = ptpool.tile(
                                    [P, 512], BF16, tag="pt2", bufs=6, name="pt2"
                                )
                                nc.vector.tensor_mul(
                                    pt2[:, c0:], pt[:, c0:],
                                    mask_sb[:, mi * 512 + c0:(mi + 1) * 512],
                                )
                                return pt2, c0
                            return pt, c0
                        return go

                    def mk_av(i, j, nm, pt_get, h=h, b=b, ostack=ostack, v_sb=v_sb):
                        def go():
                            pt, c0 = pt_get()
                            if j not in ps_os:
                                ps_os[j] = pspool.tile(
                                    [DH + 1, 512], F32, tag="po", bufs=2, name="ps_o"
                                )
                            nc.tensor.matmul(
                                ps_os[j][:, c0:],
                                lhsT=v_sb[:, (h * NT + i) * VG:(h * NT + i) * VG + DH + 1],
                                rhs=pt[:, c0:],
                                start=(i == 0),
                                stop=(i == nm - 1),
                            )
                            if i == nm - 1:
                                # normalize by the denominator (row 64):
                                # DVE only does the fast reciprocal; ACT
                                # evacuates O; Pool broadcasts + multiplies
                                # (keeps the busy DVE queue unblocked)
                                ps_o = ps_os.pop(j)
                                den = stpool.tile([1, 512], F32, tag="den", name="den")
                                nc.vector.tensor_copy(den[:], ps_o[DH:DH + 1, :])
                                rec = stpool.tile([1, 512], F32, tag="rec", name="rec")
                                nc.vector.reciprocal_approx_fast(rec[:], den[:])
                                ot = stpool.tile([DH, 512], F32, tag="ot", bufs=2, name="ot")
                                nc.scalar.activation(ot[:], ps_o[:DH, :], AF.Copy)
                                recb = stpool.tile(
                                    [DH, 512], F32, tag="recb", bufs=2, name="recb"
                                )
                                nc.gpsimd.partition_broadcast(
                                    recb[:], rec[:], channels=DH
                                )
                                nc.vector.tensor_mul(
                                    ostack[h * DH:(h + 1) * DH, j * 512:(j + 1) * 512],
                                    ot[:], recb[:],
                                )
                        return go

                    # j=0: 4 tiles (all diagonal band); j=1: 8 tiles.
                    # Score tiles run now; each A@v chases 2 tiles behind,
                    # the tail spills into `pending` for the next phase.
                    # Last batch, last head: the output-projection tiles
                    # that only need the j=0 normalizes (emitted by
                    # iteration 5 inside the A@v chase) drain inside this
                    # loop so the endgame chain stays overlapped.
                    last = (b == B - 1 and h == HL - 1)
                    b3ops = emit_outproj(b, ostack) if last else None
                    b3k = 0
                    run_q = []
                    for idx, (i, j) in enumerate(
                        [(0, 0), (1, 0), (2, 0), (3, 0)]
                        + [(i, 1) for i in range(NT)]
                    ):
                        nm = 4 if j == 0 else NT
                        pt_cell = [None]
                        sc = mk_score(i, j)

                        def run_sc(sc=sc, pt_cell=pt_cell):
                            pt_cell[0] = sc()

                        def get_pt(pt_cell=pt_cell):
                            return pt_cell[0]

                        run_sc()
                        run_q.append(mk_av(i, j, nm, get_pt))
                        if len(run_q) > 3:
                            run_q.pop(0)()
                        if b3ops is not None and idx >= 6:
                            for _ in range(2):
                                if b3k < 8:
                                    b3ops[b3k]()
                                    b3k += 1
                    pending.extend(run_q)
                    if b3ops is not None:
                        pending.extend(b3ops[8:])

                # previous batch's output projection, interleaved into the
                # next batch's t-proj phase via `pending`. For the last
                # batch: tiles 0-3 only need the j=0 normalizes, so they
                # jump ahead of the deferred j=1 A@v tail.
                if b < B - 1:
                    pending.extend(emit_outproj(b, ostack))

            for fn in pending:
                fn()
    nc.finalize()
    return nc


def _get_nc():
    if "nc" not in _CACHE:
        _CACHE["nc"] = _build_nc()
    return _CACHE["nc"]


def make_in_maps(x, Wkqv, bkqv, Wp):
    bf16 = ml_dtypes.bfloat16
    f8 = ml_dtypes.float8_e4m3
    xt = np.ascontiguousarray(np.transpose(x, (0, 2, 1)))
    xt_b = xt.astype(bf16)
    xt_8 = xt.astype(f8)
    pidx = np.arange(P)[:, None]
    fidx = np.arange(512)[None, :]
    masks = np.stack(
        [(pidx + P * i <= fidx) for i in range(4)]
    ).astype(bf16)
    in_maps = []
    for c in range(NCORES):
        w_dt = f8 if MODE >= 3 else bf16
        w_sc = MSC if MODE >= 3 else 1.0
        wqT = np.stack(
            [
                np.ascontiguousarray(Wkqv[HL * c + hh, :, D:2 * D].T) * w_sc
                for hh in range(HL)
            ]
        ).astype(w_dt)
        wkT = np.stack(
            [
                np.ascontiguousarray(Wkqv[HL * c + hh, :, :D].T) * w_sc
                for hh in range(HL)
            ]
        ).astype(w_dt)
        wv = np.ascontiguousarray(
            np.concatenate(
                [Wkqv[HL * c + hh, :, 2 * D:] for hh in range(HL)], axis=1
            )
        ).astype(bf16)
        wp = np.ascontiguousarray(Wp[P * c:P * (c + 1)]).astype(bf16)
        in_maps.append({
            "xt": xt_b, "xt8": xt_8, "wqT": wqT, "wkT": wkT, "wv": wv,
            "wp": wp, "masks": masks,
        })
    return in_maps


def run(x, Wkqv, bkqv, Wp, bp, trace=False):
    nc = _get_nc()
    in_maps = make_in_maps(x, Wkqv, bkqv, Wp)
    res = run_bass_kernel_spmd(nc, in_maps, core_ids=list(range(NCORES)), trace=trace)
    total = None
    for r in res.results:
        part = r["out"].astype(np.float64)
        total = part if total is None else total + part
    out = (total + np.asarray(bp, np.float64)).astype(np.float32)
    return out, res


def kernel(x, Wkqv, bkqv, Wp, bp):
    out, _ = run(x, Wkqv, bkqv, Wp, bp, trace=False)
    return out
